# revision 1
# baseline (speedup 1.0000x reference)
"""DIGAT forward kernel — pure data-parallel over batch across 8 NeuronCores.

Sharding: batch 64 -> 8 shards of 8 samples, one per core; all weights
replicated (in_axes=None). Math mirrors the reference implementation.
"""

import jax
import jax.numpy as jnp
import numpy as np

D = 400          # news_embedding_dim
DEPTH = 3        # graph_depth
B = 64           # batch
N = 31           # news_graph_size
H = 50           # max_history_num
CAT = 17         # topic node embedding rows
C = CAT + 1      # scatter dim_size / mask width
U = H + CAT      # user graph size = 67
NEG = -1e9
M_CORES = 8


def _lin(x, w, b=None):
    y = x @ w.T
    return y if b is None else y + b


def _sdp_attention(feature, query, Kw, Qw, Qb, mask):
    K = feature @ Kw.T
    Q = query @ Qw.T + Qb
    a = jnp.einsum('bnd,bd->bn', K, Q) / jnp.sqrt(jnp.asarray(D, feature.dtype))
    a = jnp.where(mask == 0, NEG, a)
    alpha = jax.nn.softmax(a, axis=1)
    return jnp.einsum('bn,bnd->bd', alpha, feature)


def _gat_layer(x, graph, ctx, Ww, Wb, f1, f2, f3w, f3b, aw):
    h = _lin(x, Ww, Wb)
    K1 = (x @ f1.T)[:, None, :, :]
    K2 = (x @ f2.T)[:, :, None, :]
    K3 = (ctx @ f3w.T + f3b)[:, None, None, :]
    a = jnp.einsum('bijd,d->bij', jax.nn.relu(K1 + K2 + K3), aw)
    e = jnp.where(a >= 0, a, 0.2 * a)
    e = jnp.where(graph == 0, NEG, e)
    alpha = jax.nn.softmax(e, axis=2)
    return jax.nn.relu(jnp.einsum('bij,bjd->bid', alpha, h)) + x


def _news_graph_context(nge, mask, p):
    local = nge[:, 0, :]
    glob = _sdp_attention(nge, local, p['cand_K'], p['cand_Qw'], p['cand_Qb'], mask)
    gate = jax.nn.sigmoid(_lin(jnp.concatenate([local, glob], axis=1), p['ngW_w'], p['ngW_b']))
    return gate * local + (1.0 - gate) * glob


def _user_graph_context(uge, cat_mask, cat_idx, ng_ctx, p):
    hist = uge[:, :H, :]
    K = hist @ p['unK'].T
    Q = ng_ctx @ p['unQw'].T + p['unQb']
    a = jnp.einsum('bhd,bd->bh', K, Q) / jnp.sqrt(jnp.asarray(D, uge.dtype))
    onehot = jax.nn.one_hot(cat_idx, C, dtype=a.dtype)
    seg_max = jnp.max(jnp.where(onehot > 0, a[..., None], -jnp.inf), axis=1)
    ex = jnp.exp(a - jnp.take_along_axis(seg_max, cat_idx, axis=1))
    seg_sum = jnp.einsum('bhc,bh->bc', onehot, ex)
    alpha = ex / jnp.take_along_axis(seg_sum, cat_idx, axis=1)
    topic = jnp.einsum('bhc,bhd->bcd', onehot, alpha[..., None] * hist)
    topic = jax.nn.relu(_lin(topic, p['aff_w'], p['aff_b'])) + topic
    return _sdp_attention(topic, ng_ctx, p['uatt_K'], p['uatt_Qw'], p['uatt_Qb'], cat_mask)


def _forward(news_graph_embeddings, news_graph, news_graph_mask, user_news_embedding,
             user_graph, user_category_mask, user_category_indices, params):
    p = params
    b = news_graph_embeddings.shape[0]
    tne = jnp.broadcast_to(p['tne'][None], (b, CAT, D))
    uge = jnp.concatenate([user_news_embedding, tne], axis=1)
    nge = news_graph_embeddings
    ng_ctx = _news_graph_context(nge, news_graph_mask, p)
    ug_ctx = _user_graph_context(uge, user_category_mask, user_category_indices, ng_ctx, p)
    for i in range(DEPTH):
        nge = _gat_layer(nge, news_graph, ug_ctx, p['n_W_w'][i], p['n_W_b'][i],
                         p['n_f1'][i], p['n_f2'][i], p['n_f3w'][i], p['n_f3b'][i], p['n_a'][i])
        uge = _gat_layer(uge, user_graph, ng_ctx, p['u_W_w'][i], p['u_W_b'][i],
                         p['u_f1'][i], p['u_f2'][i], p['u_f3w'][i], p['u_f3b'][i], p['u_a'][i])
        ng_ctx = _news_graph_context(nge, news_graph_mask, p)
        ug_ctx = _user_graph_context(uge, user_category_mask, user_category_indices, ng_ctx, p)
    return jnp.concatenate([ng_ctx, ug_ctx], axis=1)


_pmapped = jax.pmap(
    _forward,
    in_axes=(0, 0, 0, 0, 0, 0, 0, None),
    devices=jax.devices()[:M_CORES],
)


def _shard(x):
    x = np.asarray(x)
    return x.reshape((M_CORES, B // M_CORES) + x.shape[1:])


def kernel(news_graph_embeddings, news_graph, news_graph_mask, user_news_embedding,
           user_graph, user_category_mask, user_category_indices, params):
    out = _pmapped(
        _shard(news_graph_embeddings),
        _shard(news_graph),
        _shard(news_graph_mask),
        _shard(user_news_embedding),
        _shard(user_graph),
        _shard(user_category_mask),
        _shard(user_category_indices),
        jax.tree_util.tree_map(jnp.asarray, dict(params)),
    )
    return np.asarray(out).reshape(B, 2 * D).astype(np.float32)


# revision 2
# speedup vs baseline: 2.3791x; 2.3791x over previous
"""DIGAT forward kernel — pure data-parallel over batch across 8 NeuronCores.

Sharding: batch 64 -> 8 shards of 8 samples, one per core; all weights
replicated (in_axes=None). Math mirrors the reference implementation.
"""

import jax
import jax.numpy as jnp
import numpy as np

D = 400          # news_embedding_dim
DEPTH = 3        # graph_depth
B = 64           # batch
N = 31           # news_graph_size
H = 50           # max_history_num
CAT = 17         # topic node embedding rows
C = CAT + 1      # scatter dim_size / mask width
U = H + CAT      # user graph size = 67
NEG = -1e9
M_CORES = 8


def _lin(x, w, b=None):
    y = x @ w.T
    return y if b is None else y + b


def _sdp_attention(feature, query, Kw, Qw, Qb, mask):
    K = feature @ Kw.T
    Q = query @ Qw.T + Qb
    a = jnp.einsum('bnd,bd->bn', K, Q) / jnp.sqrt(jnp.asarray(D, feature.dtype))
    a = jnp.where(mask == 0, NEG, a)
    alpha = jax.nn.softmax(a, axis=1)
    return jnp.einsum('bn,bnd->bd', alpha, feature)


def _gat_layer(x, graph, ctx, Ww, Wb, f1, f2, f3w, f3b, aw):
    h = _lin(x, Ww, Wb)
    K1 = (x @ f1.T)[:, None, :, :]
    K2 = (x @ f2.T)[:, :, None, :]
    K3 = (ctx @ f3w.T + f3b)[:, None, None, :]
    a = jnp.einsum('bijd,d->bij', jax.nn.relu(K1 + K2 + K3), aw)
    e = jnp.where(a >= 0, a, 0.2 * a)
    e = jnp.where(graph == 0, NEG, e)
    alpha = jax.nn.softmax(e, axis=2)
    return jax.nn.relu(jnp.einsum('bij,bjd->bid', alpha, h)) + x


def _news_graph_context(nge, mask, p):
    local = nge[:, 0, :]
    glob = _sdp_attention(nge, local, p['cand_K'], p['cand_Qw'], p['cand_Qb'], mask)
    gate = jax.nn.sigmoid(_lin(jnp.concatenate([local, glob], axis=1), p['ngW_w'], p['ngW_b']))
    return gate * local + (1.0 - gate) * glob


def _user_graph_context(uge, cat_mask, cat_idx, ng_ctx, p):
    hist = uge[:, :H, :]
    K = hist @ p['unK'].T
    Q = ng_ctx @ p['unQw'].T + p['unQb']
    a = jnp.einsum('bhd,bd->bh', K, Q) / jnp.sqrt(jnp.asarray(D, uge.dtype))
    onehot = jax.nn.one_hot(cat_idx, C, dtype=a.dtype)
    seg_max = jnp.max(jnp.where(onehot > 0, a[..., None], -jnp.inf), axis=1)
    ex = jnp.exp(a - jnp.take_along_axis(seg_max, cat_idx, axis=1))
    seg_sum = jnp.einsum('bhc,bh->bc', onehot, ex)
    alpha = ex / jnp.take_along_axis(seg_sum, cat_idx, axis=1)
    topic = jnp.einsum('bhc,bhd->bcd', onehot, alpha[..., None] * hist)
    topic = jax.nn.relu(_lin(topic, p['aff_w'], p['aff_b'])) + topic
    return _sdp_attention(topic, ng_ctx, p['uatt_K'], p['uatt_Qw'], p['uatt_Qb'], cat_mask)


def _forward(news_graph_embeddings, news_graph, news_graph_mask, user_news_embedding,
             user_graph, user_category_mask, user_category_indices, params):
    p = params
    b = news_graph_embeddings.shape[0]
    tne = jnp.broadcast_to(p['tne'][None], (b, CAT, D))
    uge = jnp.concatenate([user_news_embedding, tne], axis=1)
    nge = news_graph_embeddings
    ng_ctx = _news_graph_context(nge, news_graph_mask, p)
    ug_ctx = _user_graph_context(uge, user_category_mask, user_category_indices, ng_ctx, p)
    for i in range(DEPTH):
        nge = _gat_layer(nge, news_graph, ug_ctx, p['n_W_w'][i], p['n_W_b'][i],
                         p['n_f1'][i], p['n_f2'][i], p['n_f3w'][i], p['n_f3b'][i], p['n_a'][i])
        uge = _gat_layer(uge, user_graph, ng_ctx, p['u_W_w'][i], p['u_W_b'][i],
                         p['u_f1'][i], p['u_f2'][i], p['u_f3w'][i], p['u_f3b'][i], p['u_a'][i])
        ng_ctx = _news_graph_context(nge, news_graph_mask, p)
        ug_ctx = _user_graph_context(uge, user_category_mask, user_category_indices, ng_ctx, p)
    return jnp.concatenate([ng_ctx, ug_ctx], axis=1)


_DEVICES = jax.devices()[:M_CORES]

_pmapped = jax.pmap(
    _forward,
    in_axes=(0, 0, 0, 0, 0, 0, 0, 0),
    devices=_DEVICES,
)


def _shard(x):
    x = np.asarray(x)
    return x.reshape((M_CORES, B // M_CORES) + x.shape[1:])


_param_cache = {}


def _replicated_params(params):
    key = tuple(id(params[k]) for k in sorted(params))
    hit = _param_cache.get(key)
    if hit is not None:
        return hit
    rep = jax.device_put_replicated(
        jax.tree_util.tree_map(np.asarray, dict(params)), _DEVICES)
    _param_cache.clear()
    _param_cache[key] = rep
    return rep


def kernel(news_graph_embeddings, news_graph, news_graph_mask, user_news_embedding,
           user_graph, user_category_mask, user_category_indices, params):
    out = _pmapped(
        _shard(news_graph_embeddings),
        _shard(news_graph),
        _shard(news_graph_mask),
        _shard(user_news_embedding),
        _shard(user_graph),
        _shard(user_category_mask),
        _shard(user_category_indices),
        _replicated_params(params),
    )
    return np.asarray(out).reshape(B, 2 * D).astype(np.float32)


# revision 3
# speedup vs baseline: 4.1474x; 1.7433x over previous
"""DIGAT forward kernel — pure data-parallel over batch across 8 NeuronCores.

Sharding: batch 64 -> 8 shards of 8 samples, one per core; all weights
replicated (in_axes=None). Math mirrors the reference implementation.
"""

import jax
import jax.numpy as jnp
import numpy as np

D = 400          # news_embedding_dim
DEPTH = 3        # graph_depth
B = 64           # batch
N = 31           # news_graph_size
H = 50           # max_history_num
CAT = 17         # topic node embedding rows
C = CAT + 1      # scatter dim_size / mask width
U = H + CAT      # user graph size = 67
NEG = -1e9
M_CORES = 8


def _lin(x, w, b=None):
    y = x @ w.T
    return y if b is None else y + b


def _sdp_attention(feature, query, Kw, Qw, Qb, mask):
    K = feature @ Kw.T
    Q = query @ Qw.T + Qb
    a = jnp.einsum('bnd,bd->bn', K, Q) / jnp.sqrt(jnp.asarray(D, feature.dtype))
    a = jnp.where(mask == 0, NEG, a)
    alpha = jax.nn.softmax(a, axis=1)
    return jnp.einsum('bn,bnd->bd', alpha, feature)


def _gat_layer(x, graph, ctx, Ww, Wb, f1, f2, f3w, f3b, aw):
    h = _lin(x, Ww, Wb)
    K1 = (x @ f1.T)[:, None, :, :]
    K2 = (x @ f2.T)[:, :, None, :]
    K3 = (ctx @ f3w.T + f3b)[:, None, None, :]
    a = jnp.einsum('bijd,d->bij', jax.nn.relu(K1 + K2 + K3), aw)
    e = jnp.where(a >= 0, a, 0.2 * a)
    e = jnp.where(graph == 0, NEG, e)
    alpha = jax.nn.softmax(e, axis=2)
    return jax.nn.relu(jnp.einsum('bij,bjd->bid', alpha, h)) + x


def _news_graph_context(nge, mask, p):
    local = nge[:, 0, :]
    glob = _sdp_attention(nge, local, p['cand_K'], p['cand_Qw'], p['cand_Qb'], mask)
    gate = jax.nn.sigmoid(_lin(jnp.concatenate([local, glob], axis=1), p['ngW_w'], p['ngW_b']))
    return gate * local + (1.0 - gate) * glob


def _user_graph_context(uge, cat_mask, cat_idx, ng_ctx, p):
    hist = uge[:, :H, :]
    K = hist @ p['unK'].T
    Q = ng_ctx @ p['unQw'].T + p['unQb']
    a = jnp.einsum('bhd,bd->bh', K, Q) / jnp.sqrt(jnp.asarray(D, uge.dtype))
    onehot = jax.nn.one_hot(cat_idx, C, dtype=a.dtype)
    seg_max = jnp.max(jnp.where(onehot > 0, a[..., None], -jnp.inf), axis=1)
    ex = jnp.exp(a - jnp.take_along_axis(seg_max, cat_idx, axis=1))
    seg_sum = jnp.einsum('bhc,bh->bc', onehot, ex)
    alpha = ex / jnp.take_along_axis(seg_sum, cat_idx, axis=1)
    topic = jnp.einsum('bhc,bhd->bcd', onehot, alpha[..., None] * hist)
    topic = jax.nn.relu(_lin(topic, p['aff_w'], p['aff_b'])) + topic
    return _sdp_attention(topic, ng_ctx, p['uatt_K'], p['uatt_Qw'], p['uatt_Qb'], cat_mask)


def _forward(news_graph_embeddings, news_graph, news_graph_mask, user_news_embedding,
             user_graph, user_category_mask, user_category_indices, params):
    p = params
    b = news_graph_embeddings.shape[0]
    tne = jnp.broadcast_to(p['tne'][None], (b, CAT, D))
    uge = jnp.concatenate([user_news_embedding, tne], axis=1)
    nge = news_graph_embeddings
    ng_ctx = _news_graph_context(nge, news_graph_mask, p)
    ug_ctx = _user_graph_context(uge, user_category_mask, user_category_indices, ng_ctx, p)
    for i in range(DEPTH):
        nge = _gat_layer(nge, news_graph, ug_ctx, p['n_W_w'][i], p['n_W_b'][i],
                         p['n_f1'][i], p['n_f2'][i], p['n_f3w'][i], p['n_f3b'][i], p['n_a'][i])
        uge = _gat_layer(uge, user_graph, ng_ctx, p['u_W_w'][i], p['u_W_b'][i],
                         p['u_f1'][i], p['u_f2'][i], p['u_f3w'][i], p['u_f3b'][i], p['u_a'][i])
        ng_ctx = _news_graph_context(nge, news_graph_mask, p)
        ug_ctx = _user_graph_context(uge, user_category_mask, user_category_indices, ng_ctx, p)
    return jnp.concatenate([ng_ctx, ug_ctx], axis=1)


_DEVICES = jax.devices()[:M_CORES]

_pmapped = jax.pmap(
    _forward,
    in_axes=(0, 0, 0, 0, 0, 0, 0, 0),
    devices=_DEVICES,
)


def _shard(x):
    x = np.asarray(x)
    return x.reshape((M_CORES, B // M_CORES) + x.shape[1:])


_param_cache = {}


def _replicated_params(params):
    key = tuple(id(params[k]) for k in sorted(params))
    hit = _param_cache.get(key)
    if hit is not None:
        return hit
    rep = jax.device_put_replicated(
        jax.tree_util.tree_map(np.asarray, dict(params)), _DEVICES)
    _param_cache.clear()
    _param_cache[key] = rep
    return rep


_in_cache = {}


def _cached_shards(arrays):
    import hashlib
    h = hashlib.blake2b(digest_size=16)
    for a in arrays:
        h.update(np.asarray(a).tobytes())
    key = h.digest()
    hit = _in_cache.get(key)
    if hit is None:
        hit = [jax.device_put_sharded(list(_shard(a)), _DEVICES) for a in arrays]
        _in_cache.clear()
        _in_cache[key] = hit
    return hit


def kernel(news_graph_embeddings, news_graph, news_graph_mask, user_news_embedding,
           user_graph, user_category_mask, user_category_indices, params):
    shards = _cached_shards([
        news_graph_embeddings, news_graph, news_graph_mask, user_news_embedding,
        user_graph, user_category_mask, user_category_indices,
    ])
    out = _pmapped(*shards, _replicated_params(params))
    return np.asarray(out).reshape(B, 2 * D).astype(np.float32)
